# revision 37
# baseline (speedup 1.0000x reference)
"""MultiHeadLocalAttention Trainium2 kernel.

Sharding: data-parallel over batch B=8 across the 8 NeuronCores (one batch
element per core). Each core runs the full pipeline for its element:
QKV projections, banded local attention (window 33 + cls), cls full
attention, and the output projection.

Layouts on-chip (per core):
  xT, QT, KT : [feat, f] with f = abs_token + 16 (zero-padded both sides).
  Vpp_t      : [128, 390] 65-stride V: cols 65h+0..63 = V head h for
               rows = abs tokens [128t-15, 128t+113), col 65h+64 = 1.0
               (fused softmax row-sums ride along in the AV matmul).
  Vtail2_b   : [33, 390]: rows 0..31 = Vpp rows for tokens
               [128b+113, 128b+145), row 32 = vcls (65-stride + ones).
  OT         : [feat x 3 tiles, 1025] attention output (transposed).

v5: banded attention computes scores TRANSPOSED (S^T[k, q]) so the AV
matmul consumes exp(S^T) directly -- no A-transpose step.  Per block:
18 score matmuls (2 key chunks + cls row, 6 heads), 12 AV matmuls
(n=65, sums fused via ones column), 3 output transposes.  Softmax
normalization happens on the natural-layout AV output (per-partition
reciprocal), then the normalized O transposes into OT.
"""

import os
import sys

sys.path.insert(0, "/opt/trn_rl_repo")

import numpy as np
from contextlib import ExitStack

import concourse.bass as bass
import concourse.tile as tile
from concourse import bacc, mybir

H, D = 6, 64
WIN, PAD = 33, 16
B, L, E = 8, 1025, 384
NB = 8            # 128-query blocks covering tokens 1..1024
NT = 9            # token tiles
FP = 16           # f = abs + FP for xT/QT/KT
KTW = FP + L + 16         # 1057
XTW = FP + L + 128        # 1169
SPAN = 160
VW = 6 * 65               # 390: 65-stride V layout width
F32 = mybir.dt.float32
BF = mybir.dt.bfloat16
AF = mybir.ActivationFunctionType
ALU = mybir.AluOpType

TCHUNKS = [(0, 512), (512, 512), (1024, 1)]
YCHUNKS = TCHUNKS


def host_inputs(x_b, Wq, bq, Wk, bk, Wv, bv, Wo, bo):
    """Per-core input dict (numpy). x_b is this core's [L, E] slice."""
    import ml_dtypes
    bf = ml_dtypes.bfloat16
    scale = 1.0 / np.sqrt(np.float32(D))
    wq = np.asarray(Wq, np.float32) * scale
    bq6 = (np.asarray(bq, np.float32) * scale).reshape(6, 64, 1)
    bk6 = np.asarray(bk, np.float32).reshape(6, 64, 1)
    bo_eff = (
        np.asarray(bv, np.float32) @ np.asarray(Wo, np.float32)
        + np.asarray(bo, np.float32)
    ).reshape(1, E)

    # Wv in 65-stride layout (zeros in the ones-slots), + selector row
    wvp = np.zeros((E, VW), np.float32)
    wv = np.asarray(Wv, np.float32)
    for h in range(6):
        wvp[:, 65 * h:65 * h + 64] = wv[:, 64 * h:64 * h + 64]
    vsel = np.zeros((1, VW), np.float32)
    vsel[0, 64::65] = 1.0

    # transposed band masks, head-replicated:
    #   maskT1 [3, 128, 768]: key rows 0..127 (span cols 0..127)
    #   maskT2 [3, 33, 768]:  key rows 0..31 = span cols 128..159; row 32=cls
    r = np.arange(128)[:, None]          # query row (within block)
    c = np.arange(SPAN)[None, :]         # span col (key)
    maskT1 = np.zeros((3, 128, 768), np.float32)
    maskT2 = np.zeros((3, 33, 768), np.float32)
    for v, b in ((0, 0), (1, 3), (2, NB - 1)):
        absk = 128 * b - 15 + c
        m = (c - r >= 0) & (c - r <= 32) & (absk >= 1) & (absk <= L - 1)
        mT = m.astype(np.float32).T      # [160 keys, 128 q]
        for h in range(6):
            maskT1[v, :, 128 * h:128 * h + 128] = mT[0:128]
            maskT2[v, 0:32, 128 * h:128 * h + 128] = mT[128:160]
            maskT2[v, 32, 128 * h:128 * h + 128] = 1.0
    ident = np.eye(128, dtype=np.float32)

    # x transposed + f-padded on host: [E, XTW], f = abs_token + FP
    xt = np.zeros((E, XTW), np.float32)
    xt[:, FP:FP + L] = np.asarray(x_b, np.float32).T

    # all 12 bias vectors in one [64, 12] tensor (col h: bq heads, then bk)
    smalls = np.concatenate(
        [bq6[:, :, 0].T, bk6[:, :, 0].T], axis=1
    ).astype(np.float32)

    return {
        "xt": np.ascontiguousarray(xt, dtype=bf),
        "wq": np.ascontiguousarray(wq, dtype=bf),
        "wk": np.ascontiguousarray(Wk, dtype=bf),
        "smalls": np.ascontiguousarray(smalls),
        "wvp": np.ascontiguousarray(wvp, dtype=bf),
        "vsel": np.ascontiguousarray(vsel, dtype=bf),
        "wo": np.ascontiguousarray(Wo, dtype=bf),
        "bo_row": np.ascontiguousarray(bo_eff, dtype=bf),
        "ident": np.ascontiguousarray(ident, dtype=bf),
        "maskT1": np.ascontiguousarray(maskT1, dtype=bf),
        "maskT2": np.ascontiguousarray(maskT2, dtype=bf),
    }


def build_program(nc):
    # declaration order = host->HBM staging order: x + Q/K weights + biases
    # first (phase 2), then V/O weights, then transpose ident + masks
    xd = nc.dram_tensor("xt", [E, XTW], BF, kind="ExternalInput").ap()
    wqd = nc.dram_tensor("wq", [E, E], BF, kind="ExternalInput").ap()
    wkd = nc.dram_tensor("wk", [E, E], BF, kind="ExternalInput").ap()
    smd = nc.dram_tensor("smalls", [64, 12], F32, kind="ExternalInput").ap()
    wvpd = nc.dram_tensor("wvp", [E, VW], BF, kind="ExternalInput").ap()
    vseld = nc.dram_tensor("vsel", [1, VW], BF, kind="ExternalInput").ap()
    wod = nc.dram_tensor("wo", [E, E], BF, kind="ExternalInput").ap()
    bord = nc.dram_tensor("bo_row", [1, E], BF, kind="ExternalInput").ap()
    idd = nc.dram_tensor("ident", [128, 128], BF, kind="ExternalInput").ap()
    mk1d = nc.dram_tensor("maskT1", [3, 128, 768], BF, kind="ExternalInput").ap()
    mk2d = nc.dram_tensor("maskT2", [3, 33, 768], BF, kind="ExternalInput").ap()
    outd = nc.dram_tensor("out", [L, E], BF, kind="ExternalOutput").ap()

    with tile.TileContext(nc) as tc, ExitStack() as ctx:
        singles = ctx.enter_context(tc.tile_pool(name="singles", bufs=1))
        apool = ctx.enter_context(tc.tile_pool(name="apool", bufs=2))
        opool = ctx.enter_context(tc.tile_pool(name="opool", bufs=2))
        spool = ctx.enter_context(tc.tile_pool(name="small", bufs=4))
        ypool = ctx.enter_context(tc.tile_pool(name="ypool", bufs=2))
        ps_big = ctx.enter_context(tc.tile_pool(name="ps_big", bufs=2, space="PSUM"))
        ps_st = ctx.enter_context(tc.tile_pool(name="ps_st", bufs=1, space="PSUM"))
        ps_fix = ctx.enter_context(tc.tile_pool(name="ps_fix", bufs=1, space="PSUM"))
        ps_o = ctx.enter_context(tc.tile_pool(name="ps_o", bufs=1, space="PSUM"))

        def pbig(dt=F32):
            return ps_big.tile([128, 512], dt, tag="pbig", name="pbig")

        def po_tile():
            return ps_o.tile([128, 512], F32, tag="po", name="po")

        # psum scratch bank: F32 view = 3rd rotation slot for phase 2;
        # BF halves = ping-pong transpose scratch for phases 4/5
        pt_f32 = ps_fix.tile([128, 512], F32, tag="ptpp", name="ptpp")
        nc.vector.memset(pt_f32[:], 0.0)
        pt_bf = pt_f32.bitcast(BF)
        pt_pp = [pt_bf[:, 0:256], pt_bf[:, 256:512]]

        # ---- persistent SBUF tensors ----
        xT = [singles.tile([128, XTW], BF, tag=f"xT{j}", name=f"xT{j}")
              for j in range(3)]
        QT = [singles.tile([64, KTW], BF, tag=f"QT{h}", name=f"QT{h}")
              for h in range(6)]
        KT = [singles.tile([64, KTW], BF, tag=f"KT{h}", name=f"KT{h}")
              for h in range(6)]
        OT = [singles.tile([128, L], BF, tag=f"OT{j}", name=f"OT{j}")
              for j in range(3)]
        Vpp = [singles.tile([128, VW], BF, tag=f"Vpp{t}", name=f"Vpp{t}")
               for t in range(NT)]
        Vtail = [singles.tile([33, VW], BF, tag=f"Vt{b}", name=f"Vt{b}")
                 for b in range(NB)]
        vclsp = singles.tile([1, VW], BF, tag="vclsp", name="vclsp")

        for h in range(6):
            nc.vector.memset(KT[h][:, 0:FP], 0.0)
            nc.vector.memset(KT[h][:, FP + L:KTW], 0.0)

        # ---- phase 1: x arrives pre-transposed + padded from host
        #      (dispatch split across the two hardware DGE queues) ----
        for j in range(3):
            eng = nc.sync if j == 0 else nc.scalar
            eng.dma_start(out=xT[j][:], in_=xd[j * 128:(j + 1) * 128, :])

        # weights / biases / masks stream in behind x
        wsb = {}
        for nm, dr, w, eng in (("wq", wqd, E, nc.sync), ("wk", wkd, E, nc.scalar),
                               ("wvp", wvpd, VW, nc.sync),
                               ("wo", wod, E, nc.scalar)):
            tiles = []
            for ki in range(3):
                t = singles.tile([128, w], BF, tag=f"{nm}{ki}", name=f"{nm}{ki}")
                eng.dma_start(out=t[:], in_=dr[ki * 128:(ki + 1) * 128, :])
                tiles.append(t)
            wsb[nm] = tiles
        smalls_sb = singles.tile([64, 12], F32, tag="smalls", name="smalls_sb")
        nc.sync.dma_start(out=smalls_sb[:], in_=smd[:])
        bsb = {"bq": [smalls_sb[:, h:h + 1] for h in range(6)],
               "bk": [smalls_sb[:, 6 + h:7 + h] for h in range(6)]}
        vsel_sb = singles.tile([1, VW], BF, tag="vsel", name="vsel_sb")
        nc.sync.dma_start(out=vsel_sb[:], in_=vseld[:])
        ones_sb = singles.tile([1, 128], BF, tag="onesr", name="ones_sb")
        nc.vector.memset(ones_sb[:], 1.0)
        bo_sb = singles.tile([1, E], BF, tag="bo", name="bo_sb")
        nc.sync.dma_start(out=bo_sb[:], in_=bord[:])
        ident_sb = singles.tile([128, 128], BF, tag="ident", name="ident_sb")
        nc.sync.dma_start(out=ident_sb[:], in_=idd[:])
        mk1_sb, mk2_sb = [], []
        for v in range(3):
            m1 = singles.tile([128, 768], BF, tag=f"mk1{v}", name=f"mk1{v}")
            nc.sync.dma_start(out=m1[:], in_=mk1d[v])
            mk1_sb.append(m1)
            m2 = singles.tile([33, 768], BF, tag=f"mk2{v}", name=f"mk2{v}")
            nc.scalar.dma_start(out=m2[:], in_=mk2d[v])
            mk2_sb.append(m2)

        # ---- phase 2: Q/K projections, two heads per matmul (m=128);
        #      psum rotates over 3 slots (2x pbig + the scratch bank) ----
        g = 0
        for nm, dest, bias in (("wq", QT, "bq"), ("wk", KT, "bk")):
            for j in range(3):
                for c0, w in TCHUNKS:
                    pp = pbig() if g % 3 < 2 else pt_f32
                    g += 1
                    for ki in range(3):
                        nc.tensor.matmul(
                            pp[0:128, 0:w],
                            lhsT=wsb[nm][ki][:, 128 * j:128 * j + 128],
                            rhs=xT[ki][:, FP + c0: FP + c0 + w],
                            start=(ki == 0), stop=(ki == 2),
                        )
                    nc.scalar.activation(
                        out=dest[2 * j][:, FP + c0: FP + c0 + w],
                        in_=pp[0:64, 0:w],
                        func=AF.Identity, bias=bsb[bias][2 * j][:], scale=1.0,
                    )
                    nc.vector.tensor_scalar_add(
                        dest[2 * j + 1][:, FP + c0: FP + c0 + w],
                        pp[64:128, 0:w], bsb[bias][2 * j + 1][:],
                    )

        # ---- phase 3: Vpp tiles (65-stride V + ones cols); vclsp; Vtails ----
        pvc = po_tile()
        for ki in range(3):
            nc.tensor.matmul(
                pvc[0:1, 0:VW], lhsT=xT[ki][:, FP:FP + 1],
                rhs=wsb["wvp"][ki][:, 0:VW], start=(ki == 0), stop=False,
            )
        nc.tensor.matmul(
            pvc[0:1, 0:VW], lhsT=ones_sb[0:1, 0:1], rhs=vsel_sb[:],
            start=False, stop=True,
        )
        nc.any.tensor_copy(vclsp[0:1, :], pvc[0:1, 0:VW])
        for t in range(NT):
            pv = pbig()
            for ki in range(3):
                nc.tensor.matmul(
                    pv[:, 0:VW],
                    lhsT=xT[ki][:, 128 * t + 1: 128 * t + 129],
                    rhs=wsb["wvp"][ki][:, 0:VW],
                    start=(ki == 0), stop=False,
                )
            nc.tensor.matmul(
                pv[:, 0:VW], lhsT=ones_sb[0:1, 0:128], rhs=vsel_sb[:],
                start=False, stop=True,
            )
            if t % 2 == 0:
                nc.vector.tensor_copy(Vpp[t][:, :], pv[:, 0:VW])
            else:
                nc.scalar.activation(out=Vpp[t][:, :], in_=pv[:, 0:VW],
                                     func=AF.Identity)
            if t >= 1:
                b = t - 1
                nc.gpsimd.tensor_copy(Vtail[b][0:32, :], Vpp[t][0:32, :])
                nc.gpsimd.tensor_copy(Vtail[b][32:33, :], vclsp[:])

        # ---- phase 5 pieces (interleaved below): cls query attention ----
        cls_a = singles.tile([128, L], BF, tag="cls_a", name="cls_a")
        cls_b = singles.tile([64, L], BF, tag="cls_b", name="cls_b")
        acls = singles.tile([6, FP - 1 + L + 129], BF, tag="acls", name="acls")
        aclsT = singles.tile([128, 6 * NT], BF, tag="aclsT", name="aclsT")
        nc.vector.memset(acls[:, 0:FP - 1], 0.0)
        nc.vector.memset(acls[:, FP - 1 + L:], 0.0)

        def cls_scores(ci):
            c0, w = YCHUNKS[ci]
            pa = pbig()
            pb = pbig()
            nc.vector.memset(pa[:], 0.0)
            nc.vector.memset(pb[:], 0.0)
            for h in range(6):
                dst = pa if h < 4 else pb
                base = 32 * (h % 4)
                nc.tensor.matmul(
                    dst[base:base + 1, 0:w],
                    lhsT=QT[h][0:64, FP:FP + 1],
                    rhs=KT[h][0:64, FP + c0:FP + c0 + w],
                    start=True, stop=True,
                    tile_position=(0, base),
                )
            nc.scalar.activation(out=cls_a[:, c0:c0 + w], in_=pa[:, 0:w], func=AF.Exp)
            nc.scalar.activation(out=cls_b[:, c0:c0 + w], in_=pb[0:64, 0:w],
                                 func=AF.Exp)

        def cls_gather():
            for h in range(6):
                src = cls_a if h < 4 else cls_b
                nc.sync.dma_start(
                    out=acls[h:h + 1, FP - 1:FP - 1 + L],
                    in_=src[32 * (h % 4):32 * (h % 4) + 1, :],
                )

        def cls_transposes():
            for t in range(NT):
                pt = pbig(BF)
                nc.tensor.transpose(
                    pt[0:128, 0:6], acls[0:6, 128 * t:128 * t + 128],
                    ident_sb[0:6, 0:6],
                )
                nc.any.tensor_copy(aclsT[:, 6 * t:6 * t + 6], pt[0:128, 0:6])

        def cls_finish():
            poc = pbig()
            for t in range(NT):
                nc.tensor.matmul(
                    poc[0:6, 0:VW], lhsT=aclsT[:, 6 * t:6 * t + 6],
                    rhs=Vpp[t][:], start=(t == 0), stop=(t == NT - 1),
                )
            rc = spool.tile([6, 1], F32, tag="rcls", name="rc")
            nc.vector.reciprocal(rc[:], poc[0:6, 64:65])
            ocls = singles.tile([6, VW], BF, tag="ocls", name="ocls")
            nc.vector.tensor_scalar_mul(ocls[:], poc[0:6, 0:VW], rc[:])
            for h in range(6):
                p, po = h // 2, 64 * (h % 2)
                nc.sync.dma_start(
                    out=OT[p][po:po + 64, 0:1],
                    in_=ocls[h:h + 1, 65 * h:65 * h + 64],
                )

        # ---- phase 6: output projection tile (interleaved below) ----
        def emit_out_tile(t):
            rows = min(128, L - t * 128)
            py = pbig()
            for ki in range(3):
                nc.tensor.matmul(
                    py[0:rows, 0:E],
                    lhsT=OT[ki][:, 128 * t:128 * t + rows],
                    rhs=wsb["wo"][ki][:, 0:E],
                    start=(ki == 0), stop=False,
                )
            nc.tensor.matmul(
                py[0:rows, 0:E], lhsT=ones_sb[0:1, 0:rows], rhs=bo_sb[:],
                start=False, stop=True,
            )
            ysb = ypool.tile([128, E], BF, tag="ysb", name="ysb")
            nc.any.tensor_copy(ysb[0:rows, :], py[0:rows, 0:E])
            nc.sync.dma_start(out=outd[128 * t:128 * t + rows, :], in_=ysb[0:rows, :])

        # ---- phase 4: banded blocks (S^T formulation) ----
        for b in range(NB):
            mv = 0 if b == 0 else (2 if b == NB - 1 else 1)
            st1 = [ps_st.tile([128, 384], F32, tag=f"st1{i}", name=f"st1{i}")
                   for i in range(2)]
            st2 = [ps_st.tile([33, 384], F32, tag=f"st2{i}", name=f"st2{i}")
                   for i in range(2)]
            for h in range(6):
                g, hh = h // 3, h % 3
                qs = QT[h][0:64, FP + 1 + 128 * b: FP + 129 + 128 * b]
                # S^T chunk 1: keys (-15..112 rel block) on partitions
                nc.tensor.matmul(
                    st1[g][:, 128 * hh:128 * hh + 128],
                    lhsT=KT[h][0:64, 128 * b + 1: 128 * b + 129],
                    rhs=qs, start=True, stop=True,
                )
                # S^T chunk 2: tail keys 113..144 (32 rows)
                nc.tensor.matmul(
                    st2[g][0:32, 128 * hh:128 * hh + 128],
                    lhsT=KT[h][0:64, 128 * b + 129: 128 * b + 161],
                    rhs=qs, start=True, stop=True,
                )
                # cls key -> row 32
                nc.tensor.matmul(
                    st2[g][32:33, 128 * hh:128 * hh + 128],
                    lhsT=KT[h][0:64, FP:FP + 1],
                    rhs=qs, start=True, stop=True,
                )
            a_e1 = apool.tile([128, 768], BF, tag="a_e1", name="a_e1")
            a_e2 = apool.tile([33, 768], BF, tag="a_e2", name="a_e2")
            for g in range(2):
                nc.scalar.activation(out=a_e1[:, 384 * g:384 * g + 384],
                                     in_=st1[g][:], func=AF.Exp)
                nc.scalar.activation(out=a_e2[:, 384 * g:384 * g + 384],
                                     in_=st2[g][:], func=AF.Exp)
            am1 = apool.tile([128, 768], BF, tag="am1", name="am1")
            am2 = apool.tile([33, 768], BF, tag="am2", name="am2")
            nc.vector.tensor_mul(am1[:], a_e1[:], mk1_sb[mv][:])
            nc.vector.tensor_mul(am2[:], a_e2[:], mk2_sb[mv][:])
            # AV: natural-layout output + fused row sums (ones cols)
            po_nat = po_tile()
            for h in range(6):
                nc.tensor.matmul(
                    po_nat[:, 65 * h:65 * h + 65],
                    lhsT=am1[:, 128 * h:128 * h + 128],
                    rhs=Vpp[b][:, 65 * h:65 * h + 65],
                    start=True, stop=False,
                )
                nc.tensor.matmul(
                    po_nat[:, 65 * h:65 * h + 65],
                    lhsT=am2[0:33, 128 * h:128 * h + 128],
                    rhs=Vtail[b][0:33, 65 * h:65 * h + 65],
                    start=False, stop=True,
                )
            o_u = opool.tile([128, VW], BF, tag="o_u", name="o_u")
            nc.vector.tensor_copy(o_u[:], po_nat[:, 0:VW])
            recips = spool.tile([128, 6], F32, tag="recips", name="recips")
            for h in range(6):
                nc.vector.reciprocal(recips[:, h:h + 1],
                                     po_nat[:, 65 * h + 64:65 * h + 65])
            o_n = opool.tile([128, E], BF, tag="o_n", name="o_n")
            for h in range(6):
                nc.vector.tensor_scalar_mul(
                    o_n[:, 64 * h:64 * h + 64], o_u[:, 65 * h:65 * h + 64],
                    recips[:, h:h + 1],
                )
            for j in range(3):
                pt = pt_pp[(b * 3 + j) % 2]
                nc.tensor.transpose(
                    pt[0:128, 0:128], o_n[:, 128 * j:128 * j + 128], ident_sb[:]
                )
                nc.any.tensor_copy(
                    OT[j][:, 1 + 128 * b: 129 + 128 * b], pt[:, 0:128]
                )
            if b == 0:
                cls_scores(0)
                cls_scores(1)
                cls_scores(2)
            elif b == 1:
                cls_gather()
                cls_transposes()
            elif b == 2:
                cls_finish()
            elif b == 3:
                emit_out_tile(0)
                emit_out_tile(1)
            elif b == 4:
                emit_out_tile(2)
                emit_out_tile(3)
            elif b == 5:
                emit_out_tile(4)
                emit_out_tile(5)
            elif b == 6:
                emit_out_tile(6)
            elif b == 7:
                emit_out_tile(7)
                emit_out_tile(8)

    nc.compile()
    return nc


_CACHE = {}


def get_nc():
    if "nc" not in _CACHE:
        nc = bacc.Bacc("TRN2", target_bir_lowering=False, debug=False)
        _CACHE["nc"] = build_program(nc)
    return _CACHE["nc"]


def kernel(x, Wq, bq, Wk, bk, Wv, bv, Wo, bo, _trace=False):
    from concourse.bass_utils import run_bass_kernel_spmd

    x = np.asarray(x)
    in_maps = [
        host_inputs(x[b], Wq, bq, Wk, bk, Wv, bv, Wo, bo) for b in range(B)
    ]
    nc = get_nc()
    res = run_bass_kernel_spmd(nc, in_maps, core_ids=list(range(8)), trace=_trace)
    out = np.stack(
        [np.asarray(res.results[b]["out"], dtype=np.float32) for b in range(B)],
        axis=0,
    )
    if _trace:
        return out, res
    return out


# revision 40
# speedup vs baseline: 1.0391x; 1.0391x over previous
"""MultiHeadLocalAttention Trainium2 kernel.

Sharding: data-parallel over batch B=8 across the 8 NeuronCores (one batch
element per core). Each core runs the full pipeline for its element:
QKV projections, banded local attention (window 33 + cls), cls full
attention, and the output projection.

Layouts on-chip (per core):
  xT, QT, KT : [feat, f] with f = abs_token + 16 (zero-padded both sides).
  Vpp_t      : [128, 390] 65-stride V: cols 65h+0..63 = V head h for
               rows = abs tokens [128t-15, 128t+113), col 65h+64 = 1.0
               (fused softmax row-sums ride along in the AV matmul).
  Vtail2_b   : [33, 390]: rows 0..31 = Vpp rows for tokens
               [128b+113, 128b+145), row 32 = vcls (65-stride + ones).
  OT         : [feat x 3 tiles, 1025] attention output (transposed).

v5: banded attention computes scores TRANSPOSED (S^T[k, q]) so the AV
matmul consumes exp(S^T) directly -- no A-transpose step.  Per block:
18 score matmuls (2 key chunks + cls row, 6 heads), 12 AV matmuls
(n=65, sums fused via ones column), 3 output transposes.  Softmax
normalization happens on the natural-layout AV output (per-partition
reciprocal), then the normalized O transposes into OT.
"""

import os
import sys

sys.path.insert(0, "/opt/trn_rl_repo")

import numpy as np
from contextlib import ExitStack

import concourse.bass as bass
import concourse.tile as tile
from concourse import bacc, mybir

H, D = 6, 64
WIN, PAD = 33, 16
B, L, E = 8, 1025, 384
NB = 8            # 128-query blocks covering tokens 1..1024
NT = 9            # token tiles
FP = 16           # f = abs + FP for xT/QT/KT
KTW = FP + L + 16         # 1057
XTW = FP + L + 128        # 1169
SPAN = 160
VW = 6 * 65               # 390: 65-stride V layout width
F32 = mybir.dt.float32
BF = mybir.dt.bfloat16
AF = mybir.ActivationFunctionType
ALU = mybir.AluOpType

TCHUNKS = [(0, 512), (512, 512), (1024, 1)]
YCHUNKS = TCHUNKS


def host_inputs(x_b, Wq, bq, Wk, bk, Wv, bv, Wo, bo):
    """Per-core input dict (numpy). x_b is this core's [L, E] slice."""
    import ml_dtypes
    bf = ml_dtypes.bfloat16
    scale = 1.0 / np.sqrt(np.float32(D))
    wq = np.asarray(Wq, np.float32) * scale
    bq6 = (np.asarray(bq, np.float32) * scale).reshape(6, 64, 1)
    bk6 = np.asarray(bk, np.float32).reshape(6, 64, 1)
    bo_eff = (
        np.asarray(bv, np.float32) @ np.asarray(Wo, np.float32)
        + np.asarray(bo, np.float32)
    ).reshape(1, E)

    # Wv in 65-stride layout (zeros in the ones-slots), + selector row
    wvp = np.zeros((E, VW), np.float32)
    wv = np.asarray(Wv, np.float32)
    for h in range(6):
        wvp[:, 65 * h:65 * h + 64] = wv[:, 64 * h:64 * h + 64]
    vsel = np.zeros((1, VW), np.float32)
    vsel[0, 64::65] = 1.0

    # transposed band masks, head-replicated:
    #   maskT1 [3, 128, 768]: key rows 0..127 (span cols 0..127)
    #   maskT2 [3, 33, 768]:  key rows 0..31 = span cols 128..159; row 32=cls
    r = np.arange(128)[:, None]          # query row (within block)
    c = np.arange(SPAN)[None, :]         # span col (key)
    maskT1 = np.zeros((3, 128, 768), np.float32)
    maskT2 = np.zeros((3, 33, 768), np.float32)
    for v, b in ((0, 0), (1, 3), (2, NB - 1)):
        absk = 128 * b - 15 + c
        m = (c - r >= 0) & (c - r <= 32) & (absk >= 1) & (absk <= L - 1)
        mT = m.astype(np.float32).T      # [160 keys, 128 q]
        for h in range(6):
            maskT1[v, :, 128 * h:128 * h + 128] = mT[0:128]
            maskT2[v, 0:32, 128 * h:128 * h + 128] = mT[128:160]
            maskT2[v, 32, 128 * h:128 * h + 128] = 1.0
    ident = np.eye(128, dtype=np.float32)

    # x transposed + f-padded on host: [E, XTW], f = abs_token + FP
    xt = np.zeros((E, XTW), np.float32)
    xt[:, FP:FP + L] = np.asarray(x_b, np.float32).T

    # all 12 bias vectors in one [64, 12] tensor (col h: bq heads, then bk)
    smalls = np.concatenate(
        [bq6[:, :, 0].T, bk6[:, :, 0].T], axis=1
    ).astype(np.float32)

    return {
        "xt": np.ascontiguousarray(xt, dtype=bf),
        "wq": np.ascontiguousarray(wq, dtype=bf),
        "wk": np.ascontiguousarray(Wk, dtype=bf),
        "smalls": np.ascontiguousarray(smalls),
        "wvp": np.ascontiguousarray(wvp, dtype=bf),
        "vsel": np.ascontiguousarray(vsel, dtype=bf),
        "wo": np.ascontiguousarray(Wo, dtype=bf),
        "bo_row": np.ascontiguousarray(bo_eff, dtype=bf),
        "ident": np.ascontiguousarray(ident, dtype=bf),
        "maskT1": np.ascontiguousarray(maskT1, dtype=bf),
        "maskT2": np.ascontiguousarray(maskT2, dtype=bf),
    }


def build_program(nc):
    # declaration order = host->HBM staging order: x + Q/K weights + biases
    # first (phase 2), then V/O weights, then transpose ident + masks
    xd = nc.dram_tensor("xt", [E, XTW], BF, kind="ExternalInput").ap()
    wqd = nc.dram_tensor("wq", [E, E], BF, kind="ExternalInput").ap()
    wkd = nc.dram_tensor("wk", [E, E], BF, kind="ExternalInput").ap()
    smd = nc.dram_tensor("smalls", [64, 12], F32, kind="ExternalInput").ap()
    wvpd = nc.dram_tensor("wvp", [E, VW], BF, kind="ExternalInput").ap()
    vseld = nc.dram_tensor("vsel", [1, VW], BF, kind="ExternalInput").ap()
    wod = nc.dram_tensor("wo", [E, E], BF, kind="ExternalInput").ap()
    bord = nc.dram_tensor("bo_row", [1, E], BF, kind="ExternalInput").ap()
    idd = nc.dram_tensor("ident", [128, 128], BF, kind="ExternalInput").ap()
    mk1d = nc.dram_tensor("maskT1", [3, 128, 768], BF, kind="ExternalInput").ap()
    mk2d = nc.dram_tensor("maskT2", [3, 33, 768], BF, kind="ExternalInput").ap()
    outd = nc.dram_tensor("out", [L, E], BF, kind="ExternalOutput").ap()

    with tile.TileContext(nc) as tc, ExitStack() as ctx:
        singles = ctx.enter_context(tc.tile_pool(name="singles", bufs=1))
        apool = ctx.enter_context(tc.tile_pool(name="apool", bufs=3))
        opool = ctx.enter_context(tc.tile_pool(name="opool", bufs=3))
        spool = ctx.enter_context(tc.tile_pool(name="small", bufs=4))
        ypool = ctx.enter_context(tc.tile_pool(name="ypool", bufs=2))
        ps_big = ctx.enter_context(tc.tile_pool(name="ps_big", bufs=2, space="PSUM"))
        ps_st = ctx.enter_context(tc.tile_pool(name="ps_st", bufs=1, space="PSUM"))
        ps_fix = ctx.enter_context(tc.tile_pool(name="ps_fix", bufs=1, space="PSUM"))
        ps_o = ctx.enter_context(tc.tile_pool(name="ps_o", bufs=1, space="PSUM"))

        def pbig(dt=F32):
            return ps_big.tile([128, 512], dt, tag="pbig", name="pbig")

        def po_tile():
            return ps_o.tile([128, 512], F32, tag="po", name="po")

        # psum scratch bank: F32 view = 3rd rotation slot for phase 2;
        # BF halves = ping-pong transpose scratch for phases 4/5
        pt_f32 = ps_fix.tile([128, 512], F32, tag="ptpp", name="ptpp")
        nc.vector.memset(pt_f32[:], 0.0)
        pt_bf = pt_f32.bitcast(BF)
        pt_pp = [pt_bf[:, 0:256], pt_bf[:, 256:512]]

        # ---- persistent SBUF tensors ----
        xT = [singles.tile([128, XTW], BF, tag=f"xT{j}", name=f"xT{j}")
              for j in range(3)]
        QT = [singles.tile([64, KTW], BF, tag=f"QT{h}", name=f"QT{h}")
              for h in range(6)]
        KT = [singles.tile([64, KTW], BF, tag=f"KT{h}", name=f"KT{h}")
              for h in range(6)]
        OT = [singles.tile([128, L], BF, tag=f"OT{j}", name=f"OT{j}")
              for j in range(3)]
        Vpp = [singles.tile([128, VW], BF, tag=f"Vpp{t}", name=f"Vpp{t}")
               for t in range(NT)]
        Vtail = [singles.tile([33, VW], BF, tag=f"Vt{b}", name=f"Vt{b}")
                 for b in range(NB)]
        vclsp = singles.tile([1, VW], BF, tag="vclsp", name="vclsp")

        for h in range(6):
            nc.vector.memset(KT[h][:, 0:FP], 0.0)
            nc.vector.memset(KT[h][:, FP + L:KTW], 0.0)

        # ---- phase 1: x arrives pre-transposed + padded from host
        #      (dispatch split across the two hardware DGE queues) ----
        for j in range(3):
            eng = nc.sync if j == 0 else nc.scalar
            eng.dma_start(out=xT[j][:], in_=xd[j * 128:(j + 1) * 128, :])

        # weights / biases / masks stream in behind x
        wsb = {}
        for nm, dr, w, eng in (("wq", wqd, E, nc.sync), ("wk", wkd, E, nc.scalar),
                               ("wvp", wvpd, VW, nc.sync),
                               ("wo", wod, E, nc.scalar)):
            tiles = []
            for ki in range(3):
                t = singles.tile([128, w], BF, tag=f"{nm}{ki}", name=f"{nm}{ki}")
                eng.dma_start(out=t[:], in_=dr[ki * 128:(ki + 1) * 128, :])
                tiles.append(t)
            wsb[nm] = tiles
        smalls_sb = singles.tile([64, 12], F32, tag="smalls", name="smalls_sb")
        nc.sync.dma_start(out=smalls_sb[:], in_=smd[:])
        bsb = {"bq": [smalls_sb[:, h:h + 1] for h in range(6)],
               "bk": [smalls_sb[:, 6 + h:7 + h] for h in range(6)]}
        vsel_sb = singles.tile([1, VW], BF, tag="vsel", name="vsel_sb")
        nc.sync.dma_start(out=vsel_sb[:], in_=vseld[:])
        ones_sb = singles.tile([1, 128], BF, tag="onesr", name="ones_sb")
        nc.vector.memset(ones_sb[:], 1.0)
        bo_sb = singles.tile([1, E], BF, tag="bo", name="bo_sb")
        nc.sync.dma_start(out=bo_sb[:], in_=bord[:])
        ident_sb = singles.tile([128, 128], BF, tag="ident", name="ident_sb")
        nc.sync.dma_start(out=ident_sb[:], in_=idd[:])
        mk1_sb, mk2_sb = [], []
        for v in range(3):
            m1 = singles.tile([128, 768], BF, tag=f"mk1{v}", name=f"mk1{v}")
            nc.sync.dma_start(out=m1[:], in_=mk1d[v])
            mk1_sb.append(m1)
            m2 = singles.tile([33, 768], BF, tag=f"mk2{v}", name=f"mk2{v}")
            nc.scalar.dma_start(out=m2[:], in_=mk2d[v])
            mk2_sb.append(m2)

        # ---- phase 2: Q/K projections, two heads per matmul (m=128);
        #      psum rotates over 3 slots (2x pbig + the scratch bank) ----
        g = 0
        for nm, dest, bias in (("wq", QT, "bq"), ("wk", KT, "bk")):
            for j in range(3):
                for c0, w in TCHUNKS:
                    pp = pbig() if g % 3 < 2 else pt_f32
                    g += 1
                    for ki in range(3):
                        nc.tensor.matmul(
                            pp[0:128, 0:w],
                            lhsT=wsb[nm][ki][:, 128 * j:128 * j + 128],
                            rhs=xT[ki][:, FP + c0: FP + c0 + w],
                            start=(ki == 0), stop=(ki == 2),
                        )
                    nc.scalar.activation(
                        out=dest[2 * j][:, FP + c0: FP + c0 + w],
                        in_=pp[0:64, 0:w],
                        func=AF.Identity, bias=bsb[bias][2 * j][:], scale=1.0,
                    )
                    nc.vector.tensor_scalar_add(
                        dest[2 * j + 1][:, FP + c0: FP + c0 + w],
                        pp[64:128, 0:w], bsb[bias][2 * j + 1][:],
                    )

        # ---- phase 3: Vpp tiles (65-stride V + ones cols); vclsp; Vtails.
        #      The ones columns are added during the psum->SBUF copy via a
        #      broadcast selector row (built once with one matmul). ----
        psel = po_tile()
        nc.tensor.matmul(psel[0:128, 0:VW], lhsT=ones_sb[0:1, 0:128],
                         rhs=vsel_sb[:], start=True, stop=True)
        vsel128 = singles.tile([128, VW], BF, tag="vsel128", name="vsel128")
        nc.any.tensor_copy(vsel128[:], psel[0:128, 0:VW])
        pvc = po_tile()
        for ki in range(3):
            nc.tensor.matmul(
                pvc[0:1, 0:VW], lhsT=xT[ki][:, FP:FP + 1],
                rhs=wsb["wvp"][ki][:, 0:VW], start=(ki == 0), stop=(ki == 2),
            )
        nc.vector.scalar_tensor_tensor(
            out=vclsp[0:1, :], in0=pvc[0:1, 0:VW], scalar=1.0,
            in1=vsel128[0:1, :], op0=ALU.mult, op1=ALU.add,
        )
        for t in range(NT):
            pv = pbig()
            for ki in range(3):
                nc.tensor.matmul(
                    pv[:, 0:VW],
                    lhsT=xT[ki][:, 128 * t + 1: 128 * t + 129],
                    rhs=wsb["wvp"][ki][:, 0:VW],
                    start=(ki == 0), stop=(ki == 2),
                )
            nc.vector.scalar_tensor_tensor(
                out=Vpp[t][:, :], in0=pv[:, 0:VW], scalar=1.0,
                in1=vsel128[:], op0=ALU.mult, op1=ALU.add,
            )
            if t >= 1:
                b = t - 1
                nc.any.tensor_copy(Vtail[b][0:32, :], Vpp[t][0:32, :])
                nc.any.tensor_copy(Vtail[b][32:33, :], vclsp[:])

        # ---- phase 5 pieces (interleaved below): cls query attention ----
        cls_a = singles.tile([128, L], BF, tag="cls_a", name="cls_a")
        cls_b = singles.tile([64, L], BF, tag="cls_b", name="cls_b")
        acls = singles.tile([6, FP - 1 + L + 129], BF, tag="acls", name="acls")
        aclsT = singles.tile([128, 6 * NT], BF, tag="aclsT", name="aclsT")
        nc.vector.memset(acls[:, 0:FP - 1], 0.0)
        nc.vector.memset(acls[:, FP - 1 + L:], 0.0)

        def cls_scores(ci):
            c0, w = YCHUNKS[ci]
            pa = pbig()
            pb = pbig()
            nc.vector.memset(pa[:], 0.0)
            nc.vector.memset(pb[:], 0.0)
            for h in range(6):
                dst = pa if h < 4 else pb
                base = 32 * (h % 4)
                nc.tensor.matmul(
                    dst[base:base + 1, 0:w],
                    lhsT=QT[h][0:64, FP:FP + 1],
                    rhs=KT[h][0:64, FP + c0:FP + c0 + w],
                    start=True, stop=True,
                    tile_position=(0, base),
                )
            nc.scalar.activation(out=cls_a[:, c0:c0 + w], in_=pa[:, 0:w], func=AF.Exp)
            nc.scalar.activation(out=cls_b[:, c0:c0 + w], in_=pb[0:64, 0:w],
                                 func=AF.Exp)

        def cls_gather():
            for h in range(6):
                src = cls_a if h < 4 else cls_b
                nc.sync.dma_start(
                    out=acls[h:h + 1, FP - 1:FP - 1 + L],
                    in_=src[32 * (h % 4):32 * (h % 4) + 1, :],
                )

        def cls_transposes():
            for t in range(NT):
                pt = pbig(BF)
                nc.tensor.transpose(
                    pt[0:128, 0:6], acls[0:6, 128 * t:128 * t + 128],
                    ident_sb[0:6, 0:6],
                )
                nc.any.tensor_copy(aclsT[:, 6 * t:6 * t + 6], pt[0:128, 0:6])

        def cls_finish():
            poc = pbig()
            for t in range(NT):
                nc.tensor.matmul(
                    poc[0:6, 0:VW], lhsT=aclsT[:, 6 * t:6 * t + 6],
                    rhs=Vpp[t][:], start=(t == 0), stop=(t == NT - 1),
                )
            rc = spool.tile([6, 1], F32, tag="rcls", name="rc")
            nc.vector.reciprocal(rc[:], poc[0:6, 64:65])
            ocls = singles.tile([6, VW], BF, tag="ocls", name="ocls")
            nc.vector.tensor_scalar_mul(ocls[:], poc[0:6, 0:VW], rc[:])
            for h in range(6):
                p, po = h // 2, 64 * (h % 2)
                nc.sync.dma_start(
                    out=OT[p][po:po + 64, 0:1],
                    in_=ocls[h:h + 1, 65 * h:65 * h + 64],
                )

        # ---- phase 6: output projection tile (interleaved below) ----
        def emit_out_tile(t):
            rows = min(128, L - t * 128)
            py = pbig()
            for ki in range(3):
                nc.tensor.matmul(
                    py[0:rows, 0:E],
                    lhsT=OT[ki][:, 128 * t:128 * t + rows],
                    rhs=wsb["wo"][ki][:, 0:E],
                    start=(ki == 0), stop=False,
                )
            nc.tensor.matmul(
                py[0:rows, 0:E], lhsT=ones_sb[0:1, 0:rows], rhs=bo_sb[:],
                start=False, stop=True,
            )
            ysb = ypool.tile([128, E], BF, tag="ysb", name="ysb")
            nc.any.tensor_copy(ysb[0:rows, :], py[0:rows, 0:E])
            nc.sync.dma_start(out=outd[128 * t:128 * t + rows, :], in_=ysb[0:rows, :])

        # ---- phase 4: banded blocks (S^T formulation) ----
        for b in range(NB):
            mv = 0 if b == 0 else (2 if b == NB - 1 else 1)
            st1 = [ps_st.tile([128, 384], F32, tag=f"st1{i}", name=f"st1{i}")
                   for i in range(2)]
            st2 = [ps_st.tile([33, 384], F32, tag=f"st2{i}", name=f"st2{i}")
                   for i in range(2)]
            for h in range(6):
                g, hh = h // 3, h % 3
                qs = QT[h][0:64, FP + 1 + 128 * b: FP + 129 + 128 * b]
                # S^T chunk 1: keys (-15..112 rel block) on partitions
                nc.tensor.matmul(
                    st1[g][:, 128 * hh:128 * hh + 128],
                    lhsT=KT[h][0:64, 128 * b + 1: 128 * b + 129],
                    rhs=qs, start=True, stop=True,
                )
                # S^T chunk 2: tail keys 113..144 (32 rows)
                nc.tensor.matmul(
                    st2[g][0:32, 128 * hh:128 * hh + 128],
                    lhsT=KT[h][0:64, 128 * b + 129: 128 * b + 161],
                    rhs=qs, start=True, stop=True,
                )
                # cls key -> row 32
                nc.tensor.matmul(
                    st2[g][32:33, 128 * hh:128 * hh + 128],
                    lhsT=KT[h][0:64, FP:FP + 1],
                    rhs=qs, start=True, stop=True,
                )
            a_e1 = apool.tile([128, 768], BF, tag="a_e1", name="a_e1")
            a_e2 = apool.tile([33, 768], BF, tag="a_e2", name="a_e2")
            for g in range(2):
                nc.scalar.activation(out=a_e1[:, 384 * g:384 * g + 384],
                                     in_=st1[g][:], func=AF.Exp)
                nc.scalar.activation(out=a_e2[:, 384 * g:384 * g + 384],
                                     in_=st2[g][:], func=AF.Exp)
            am1 = apool.tile([128, 768], BF, tag="am1", name="am1")
            am2 = apool.tile([33, 768], BF, tag="am2", name="am2")
            nc.vector.tensor_mul(am1[:], a_e1[:], mk1_sb[mv][:])
            nc.vector.tensor_mul(am2[:], a_e2[:], mk2_sb[mv][:])
            # AV: natural-layout output + fused row sums (ones cols)
            po_nat = po_tile()
            for h in range(6):
                nc.tensor.matmul(
                    po_nat[:, 65 * h:65 * h + 65],
                    lhsT=am1[:, 128 * h:128 * h + 128],
                    rhs=Vpp[b][:, 65 * h:65 * h + 65],
                    start=True, stop=False,
                )
                nc.tensor.matmul(
                    po_nat[:, 65 * h:65 * h + 65],
                    lhsT=am2[0:33, 128 * h:128 * h + 128],
                    rhs=Vtail[b][0:33, 65 * h:65 * h + 65],
                    start=False, stop=True,
                )
            o_u = opool.tile([128, VW], BF, tag="o_u", name="o_u")
            nc.vector.tensor_copy(o_u[:], po_nat[:, 0:VW])
            recips = spool.tile([128, 6], F32, tag="recips", name="recips")
            for h in range(6):
                nc.vector.reciprocal(recips[:, h:h + 1],
                                     po_nat[:, 65 * h + 64:65 * h + 65])
            o_n = opool.tile([128, E], BF, tag="o_n", name="o_n")
            for h in range(6):
                nc.vector.tensor_scalar_mul(
                    o_n[:, 64 * h:64 * h + 64], o_u[:, 65 * h:65 * h + 64],
                    recips[:, h:h + 1],
                )
            for j in range(3):
                pt = pt_pp[(b * 3 + j) % 2]
                nc.tensor.transpose(
                    pt[0:128, 0:128], o_n[:, 128 * j:128 * j + 128], ident_sb[:]
                )
                nc.any.tensor_copy(
                    OT[j][:, 1 + 128 * b: 129 + 128 * b], pt[:, 0:128]
                )
            if b == 0:
                cls_scores(0)
                cls_scores(1)
                cls_scores(2)
            elif b == 1:
                cls_gather()
                cls_transposes()
            elif b == 2:
                cls_finish()
            elif b == 3:
                emit_out_tile(0)
                emit_out_tile(1)
            elif b == 4:
                emit_out_tile(2)
                emit_out_tile(3)
            elif b == 5:
                emit_out_tile(4)
                emit_out_tile(5)
            elif b == 6:
                emit_out_tile(6)
            elif b == 7:
                emit_out_tile(7)
                emit_out_tile(8)

    nc.compile()
    return nc


_CACHE = {}


def get_nc():
    if "nc" not in _CACHE:
        nc = bacc.Bacc("TRN2", target_bir_lowering=False, debug=False)
        _CACHE["nc"] = build_program(nc)
    return _CACHE["nc"]


def kernel(x, Wq, bq, Wk, bk, Wv, bv, Wo, bo, _trace=False):
    from concourse.bass_utils import run_bass_kernel_spmd

    x = np.asarray(x)
    in_maps = [
        host_inputs(x[b], Wq, bq, Wk, bk, Wv, bv, Wo, bo) for b in range(B)
    ]
    nc = get_nc()
    res = run_bass_kernel_spmd(nc, in_maps, core_ids=list(range(8)), trace=_trace)
    out = np.stack(
        [np.asarray(res.results[b]["out"], dtype=np.float32) for b in range(B)],
        axis=0,
    )
    if _trace:
        return out, res
    return out


# revision 47
# speedup vs baseline: 1.0723x; 1.0320x over previous
"""MultiHeadLocalAttention Trainium2 kernel.

Sharding: data-parallel over batch B=8 across the 8 NeuronCores (one batch
element per core). Each core runs the full pipeline for its element:
QKV projections, banded local attention (window 33 + cls), cls full
attention, and the output projection.

Layouts on-chip (per core):
  xT, QT, KT : [feat, f] with f = abs_token + 16 (zero-padded both sides).
  Vpp_t      : [128, 390] 65-stride V: cols 65h+0..63 = V head h for
               rows = abs tokens [128t-15, 128t+113), col 65h+64 = 1.0
               (fused softmax row-sums ride along in the AV matmul).
  Vtail2_b   : [33, 390]: rows 0..31 = Vpp rows for tokens
               [128b+113, 128b+145), row 32 = vcls (65-stride + ones).
  OT         : [feat x 3 tiles, 1025] attention output (transposed).

v5: banded attention computes scores TRANSPOSED (S^T[k, q]) so the AV
matmul consumes exp(S^T) directly -- no A-transpose step.  Per block:
18 score matmuls (2 key chunks + cls row, 6 heads), 12 AV matmuls
(n=65, sums fused via ones column), 3 output transposes.  Softmax
normalization happens on the natural-layout AV output (per-partition
reciprocal), then the normalized O transposes into OT.
"""

import os
import sys

sys.path.insert(0, "/opt/trn_rl_repo")

import numpy as np
from contextlib import ExitStack

import concourse.bass as bass
import concourse.tile as tile
from concourse import bacc, mybir

H, D = 6, 64
WIN, PAD = 33, 16
B, L, E = 8, 1025, 384
NB = 8            # 128-query blocks covering tokens 1..1024
NT = 9            # token tiles
FP = 16           # f = abs + FP for xT/QT/KT
KTW = FP + L + 16         # 1057
XTW = FP + L + 128        # 1169
SPAN = 160
VW = 6 * 65               # 390: 65-stride V layout width
F32 = mybir.dt.float32
BF = mybir.dt.bfloat16
AF = mybir.ActivationFunctionType
ALU = mybir.AluOpType

TCHUNKS = [(0, 512), (512, 512), (1024, 1)]
YCHUNKS = TCHUNKS


def host_inputs(x_b, Wq, bq, Wk, bk, Wv, bv, Wo, bo):
    """Per-core input dict (numpy). x_b is this core's [L, E] slice."""
    import ml_dtypes
    bf = ml_dtypes.bfloat16
    scale = 1.0 / np.sqrt(np.float32(D))
    wq = np.asarray(Wq, np.float32) * scale
    bq6 = (np.asarray(bq, np.float32) * scale).reshape(6, 64, 1)
    bk6 = np.asarray(bk, np.float32).reshape(6, 64, 1)
    bo_eff = (
        np.asarray(bv, np.float32) @ np.asarray(Wo, np.float32)
        + np.asarray(bo, np.float32)
    ).reshape(1, E)

    # Wv in 65-stride layout (zeros in the ones-slots), + selector row
    wvp = np.zeros((E, VW), np.float32)
    wv = np.asarray(Wv, np.float32)
    for h in range(6):
        wvp[:, 65 * h:65 * h + 64] = wv[:, 64 * h:64 * h + 64]
    vsel = np.zeros((1, VW), np.float32)
    vsel[0, 64::65] = 1.0

    # transposed band masks, head-replicated:
    #   maskT1 [3, 128, 768]: key rows 0..127 (span cols 0..127)
    #   maskT2 [3, 33, 768]:  key rows 0..31 = span cols 128..159; row 32=cls
    r = np.arange(128)[:, None]          # query row (within block)
    c = np.arange(SPAN)[None, :]         # span col (key)
    maskT1 = np.zeros((3, 128, 768), np.float32)
    maskT2 = np.zeros((3, 33, 768), np.float32)
    for v, b in ((0, 0), (1, 3), (2, NB - 1)):
        absk = 128 * b - 15 + c
        m = (c - r >= 0) & (c - r <= 32) & (absk >= 1) & (absk <= L - 1)
        mT = m.astype(np.float32).T      # [160 keys, 128 q]
        for h in range(6):
            maskT1[v, :, 128 * h:128 * h + 128] = mT[0:128]
            maskT2[v, 0:32, 128 * h:128 * h + 128] = mT[128:160]
            maskT2[v, 32, 128 * h:128 * h + 128] = 1.0
    ident = np.eye(128, dtype=np.float32)

    # x transposed + f-padded on host: [E, XTW], f = abs_token + FP
    xt = np.zeros((E, XTW), np.float32)
    xt[:, FP:FP + L] = np.asarray(x_b, np.float32).T

    # all 12 bias vectors in one [64, 12] tensor (col h: bq heads, then bk)
    smalls = np.concatenate(
        [bq6[:, :, 0].T, bk6[:, :, 0].T], axis=1
    ).astype(np.float32)

    return {
        "xt": np.ascontiguousarray(xt, dtype=bf),
        "wq": np.ascontiguousarray(wq, dtype=bf),
        "wk": np.ascontiguousarray(Wk, dtype=bf),
        "smalls": np.ascontiguousarray(smalls),
        "wvp": np.ascontiguousarray(wvp, dtype=bf),
        "vsel": np.ascontiguousarray(vsel, dtype=bf),
        "wo": np.ascontiguousarray(Wo, dtype=bf),
        "bo_row": np.ascontiguousarray(bo_eff, dtype=bf),
        "ident": np.ascontiguousarray(ident, dtype=bf),
        "maskT1": np.ascontiguousarray(maskT1, dtype=bf),
        "maskT2": np.ascontiguousarray(maskT2, dtype=bf),
    }


def build_program(nc):
    # declaration order = host->HBM staging order: x + Q/K weights + biases
    # first (phase 2), then V/O weights, then transpose ident + masks
    xd = nc.dram_tensor("xt", [E, XTW], BF, kind="ExternalInput").ap()
    wqd = nc.dram_tensor("wq", [E, E], BF, kind="ExternalInput").ap()
    wkd = nc.dram_tensor("wk", [E, E], BF, kind="ExternalInput").ap()
    smd = nc.dram_tensor("smalls", [64, 12], F32, kind="ExternalInput").ap()
    wvpd = nc.dram_tensor("wvp", [E, VW], BF, kind="ExternalInput").ap()
    vseld = nc.dram_tensor("vsel", [1, VW], BF, kind="ExternalInput").ap()
    wod = nc.dram_tensor("wo", [E, E], BF, kind="ExternalInput").ap()
    bord = nc.dram_tensor("bo_row", [1, E], BF, kind="ExternalInput").ap()
    idd = nc.dram_tensor("ident", [128, 128], BF, kind="ExternalInput").ap()
    mk1d = nc.dram_tensor("maskT1", [3, 128, 768], BF, kind="ExternalInput").ap()
    mk2d = nc.dram_tensor("maskT2", [3, 33, 768], BF, kind="ExternalInput").ap()
    outd = nc.dram_tensor("out", [L, E], BF, kind="ExternalOutput").ap()

    with tile.TileContext(nc) as tc, ExitStack() as ctx:
        singles = ctx.enter_context(tc.tile_pool(name="singles", bufs=1))
        apool = ctx.enter_context(tc.tile_pool(name="apool", bufs=3))
        opool = ctx.enter_context(tc.tile_pool(name="opool", bufs=3))
        spool = ctx.enter_context(tc.tile_pool(name="small", bufs=4))
        ypool = ctx.enter_context(tc.tile_pool(name="ypool", bufs=2))
        ps_big = ctx.enter_context(tc.tile_pool(name="ps_big", bufs=2, space="PSUM"))
        ps_st = ctx.enter_context(tc.tile_pool(name="ps_st", bufs=1, space="PSUM"))
        ps_fix = ctx.enter_context(tc.tile_pool(name="ps_fix", bufs=1, space="PSUM"))
        ps_o = ctx.enter_context(tc.tile_pool(name="ps_o", bufs=1, space="PSUM"))

        def pbig(dt=F32):
            return ps_big.tile([128, 512], dt, tag="pbig", name="pbig")

        def po_tile():
            return ps_o.tile([128, 512], F32, tag="po", name="po")

        # psum scratch bank: F32 view = 3rd rotation slot for phase 2;
        # BF halves = ping-pong transpose scratch for phases 4/5
        pt_f32 = ps_fix.tile([128, 512], F32, tag="ptpp", name="ptpp")
        nc.vector.memset(pt_f32[:], 0.0)
        pt_bf = pt_f32.bitcast(BF)
        pt_pp = [pt_bf[:, 0:256], pt_bf[:, 256:512]]

        # ---- persistent SBUF tensors ----
        xT = [singles.tile([128, XTW], BF, tag=f"xT{j}", name=f"xT{j}")
              for j in range(3)]
        QT = [singles.tile([64, KTW], BF, tag=f"QT{h}", name=f"QT{h}")
              for h in range(6)]
        KT = [singles.tile([64, KTW], BF, tag=f"KT{h}", name=f"KT{h}")
              for h in range(6)]
        OT = [singles.tile([128, L], BF, tag=f"OT{j}", name=f"OT{j}")
              for j in range(3)]
        Vpp = [singles.tile([128, VW], BF, tag=f"Vpp{t}", name=f"Vpp{t}")
               for t in range(NT)]
        Vtail = [singles.tile([33, VW], BF, tag=f"Vt{b}", name=f"Vt{b}")
                 for b in range(NB)]
        vclsp = singles.tile([1, VW], BF, tag="vclsp", name="vclsp")

        for h in range(6):
            nc.vector.memset(KT[h][:, 0:FP], 0.0)
            nc.vector.memset(KT[h][:, FP + L:KTW], 0.0)

        # ---- phase 1: x arrives pre-transposed + padded from host
        #      (dispatch split across the two hardware DGE queues) ----
        for j in range(3):
            eng = nc.sync if j == 0 else nc.scalar
            eng.dma_start(out=xT[j][:], in_=xd[j * 128:(j + 1) * 128, :])

        # weights / biases / masks stream in behind x
        wsb = {}
        for nm, dr, w, eng in (("wq", wqd, E, nc.sync), ("wk", wkd, E, nc.scalar),
                               ("wvp", wvpd, VW, nc.sync),
                               ("wo", wod, E, nc.scalar)):
            tiles = []
            for ki in range(3):
                t = singles.tile([128, w], BF, tag=f"{nm}{ki}", name=f"{nm}{ki}")
                eng.dma_start(out=t[:], in_=dr[ki * 128:(ki + 1) * 128, :])
                tiles.append(t)
            wsb[nm] = tiles
        smalls_sb = singles.tile([64, 12], F32, tag="smalls", name="smalls_sb")
        nc.sync.dma_start(out=smalls_sb[:], in_=smd[:])
        bsb = {"bq": [smalls_sb[:, h:h + 1] for h in range(6)],
               "bk": [smalls_sb[:, 6 + h:7 + h] for h in range(6)]}
        vsel_sb = singles.tile([1, VW], BF, tag="vsel", name="vsel_sb")
        nc.sync.dma_start(out=vsel_sb[:], in_=vseld[:])
        ones_sb = singles.tile([1, 128], BF, tag="onesr", name="ones_sb")
        nc.vector.memset(ones_sb[:], 1.0)
        bo_sb = singles.tile([1, E], BF, tag="bo", name="bo_sb")
        nc.sync.dma_start(out=bo_sb[:], in_=bord[:])
        ident_sb = singles.tile([128, 128], BF, tag="ident", name="ident_sb")
        nc.sync.dma_start(out=ident_sb[:], in_=idd[:])
        mk1_sb, mk2_sb = [], []
        for v in range(3):
            m1 = singles.tile([128, 768], BF, tag=f"mk1{v}", name=f"mk1{v}")
            nc.sync.dma_start(out=m1[:], in_=mk1d[v])
            mk1_sb.append(m1)
            m2 = singles.tile([33, 768], BF, tag=f"mk2{v}", name=f"mk2{v}")
            nc.scalar.dma_start(out=m2[:], in_=mk2d[v])
            mk2_sb.append(m2)

        # ---- phase 2: Q/K projections, two heads per matmul (m=128);
        #      psum rotates over 3 slots (2x pbig + the scratch bank) ----
        g = 0
        for nm, dest, bias in (("wq", QT, "bq"), ("wk", KT, "bk")):
            for j in range(3):
                for c0, w in TCHUNKS:
                    pp = [pbig, pbig, lambda: pt_f32, po_tile][g % 4]()
                    g += 1
                    for ki in range(3):
                        nc.tensor.matmul(
                            pp[0:128, 0:w],
                            lhsT=wsb[nm][ki][:, 128 * j:128 * j + 128],
                            rhs=xT[ki][:, FP + c0: FP + c0 + w],
                            start=(ki == 0), stop=(ki == 2),
                        )
                    nc.scalar.activation(
                        out=dest[2 * j][:, FP + c0: FP + c0 + w],
                        in_=pp[0:64, 0:w],
                        func=AF.Identity, bias=bsb[bias][2 * j][:], scale=1.0,
                    )
                    nc.vector.tensor_scalar_add(
                        dest[2 * j + 1][:, FP + c0: FP + c0 + w],
                        pp[64:128, 0:w], bsb[bias][2 * j + 1][:],
                    )

        # ---- phase 3: Vpp tiles (65-stride V + ones cols); vclsp; Vtails.
        #      The ones columns are added during the psum->SBUF copy via a
        #      broadcast selector row (built once with one matmul). ----
        psel = po_tile()
        nc.tensor.matmul(psel[0:128, 0:VW], lhsT=ones_sb[0:1, 0:128],
                         rhs=vsel_sb[:], start=True, stop=True)
        vsel128 = singles.tile([128, VW], BF, tag="vsel128", name="vsel128")
        nc.any.tensor_copy(vsel128[:], psel[0:128, 0:VW])
        pvc = po_tile()
        for ki in range(3):
            nc.tensor.matmul(
                pvc[0:1, 0:VW], lhsT=xT[ki][:, FP:FP + 1],
                rhs=wsb["wvp"][ki][:, 0:VW], start=(ki == 0), stop=(ki == 2),
            )
        nc.vector.scalar_tensor_tensor(
            out=vclsp[0:1, :], in0=pvc[0:1, 0:VW], scalar=1.0,
            in1=vsel128[0:1, :], op0=ALU.mult, op1=ALU.add,
        )
        for t in range(NT):
            pv = pbig()
            for ki in range(3):
                nc.tensor.matmul(
                    pv[:, 0:VW],
                    lhsT=xT[ki][:, 128 * t + 1: 128 * t + 129],
                    rhs=wsb["wvp"][ki][:, 0:VW],
                    start=(ki == 0), stop=(ki == 2),
                )
            nc.vector.scalar_tensor_tensor(
                out=Vpp[t][:, :], in0=pv[:, 0:VW], scalar=1.0,
                in1=vsel128[:], op0=ALU.mult, op1=ALU.add,
            )
            if t >= 1:
                b = t - 1
                nc.any.tensor_copy(Vtail[b][0:32, :], Vpp[t][0:32, :])
                nc.any.tensor_copy(Vtail[b][32:33, :], vclsp[:])

        # ---- phase 5 pieces (interleaved below): cls query attention ----
        cls_a = singles.tile([128, L], BF, tag="cls_a", name="cls_a")
        cls_b = singles.tile([64, L], BF, tag="cls_b", name="cls_b")
        acls = singles.tile([6, FP - 1 + L + 129], BF, tag="acls", name="acls")
        aclsT = singles.tile([128, 6 * NT], BF, tag="aclsT", name="aclsT")
        nc.vector.memset(acls[:, 0:FP - 1], 0.0)
        nc.vector.memset(acls[:, FP - 1 + L:], 0.0)

        def cls_scores(ci):
            c0, w = YCHUNKS[ci]
            pa = pbig()
            pb = pbig()
            nc.vector.memset(pa[:], 0.0)
            nc.vector.memset(pb[:], 0.0)
            for h in range(6):
                dst = pa if h < 4 else pb
                base = 32 * (h % 4)
                nc.tensor.matmul(
                    dst[base:base + 1, 0:w],
                    lhsT=QT[h][0:64, FP:FP + 1],
                    rhs=KT[h][0:64, FP + c0:FP + c0 + w],
                    start=True, stop=True,
                    tile_position=(0, base),
                )
            nc.scalar.activation(out=cls_a[:, c0:c0 + w], in_=pa[:, 0:w], func=AF.Exp)
            nc.scalar.activation(out=cls_b[:, c0:c0 + w], in_=pb[0:64, 0:w],
                                 func=AF.Exp)

        def cls_gather():
            for h in range(6):
                src = cls_a if h < 4 else cls_b
                nc.sync.dma_start(
                    out=acls[h:h + 1, FP - 1:FP - 1 + L],
                    in_=src[32 * (h % 4):32 * (h % 4) + 1, :],
                )

        def cls_transposes():
            for t in range(NT):
                pt = pbig(BF)
                nc.tensor.transpose(
                    pt[0:128, 0:6], acls[0:6, 128 * t:128 * t + 128],
                    ident_sb[0:6, 0:6],
                )
                nc.any.tensor_copy(aclsT[:, 6 * t:6 * t + 6], pt[0:128, 0:6])

        def cls_finish():
            poc = pbig()
            for t in range(NT):
                nc.tensor.matmul(
                    poc[0:6, 0:VW], lhsT=aclsT[:, 6 * t:6 * t + 6],
                    rhs=Vpp[t][:], start=(t == 0), stop=(t == NT - 1),
                )
            rc = spool.tile([6, 1], F32, tag="rcls", name="rc")
            nc.vector.reciprocal(rc[:], poc[0:6, 64:65])
            ocls = singles.tile([6, VW], BF, tag="ocls", name="ocls")
            nc.vector.tensor_scalar_mul(ocls[:], poc[0:6, 0:VW], rc[:])
            for h in range(6):
                p, po = h // 2, 64 * (h % 2)
                nc.sync.dma_start(
                    out=OT[p][po:po + 64, 0:1],
                    in_=ocls[h:h + 1, 65 * h:65 * h + 64],
                )

        # ---- phase 6: output projection tile (interleaved below) ----
        def emit_out_tile(t):
            rows = min(128, L - t * 128)
            py = pbig()
            for ki in range(3):
                nc.tensor.matmul(
                    py[0:rows, 0:E],
                    lhsT=OT[ki][:, 128 * t:128 * t + rows],
                    rhs=wsb["wo"][ki][:, 0:E],
                    start=(ki == 0), stop=False,
                )
            nc.tensor.matmul(
                py[0:rows, 0:E], lhsT=ones_sb[0:1, 0:rows], rhs=bo_sb[:],
                start=False, stop=True,
            )
            ysb = ypool.tile([128, E], BF, tag="ysb", name="ysb")
            nc.any.tensor_copy(ysb[0:rows, :], py[0:rows, 0:E])
            nc.sync.dma_start(out=outd[128 * t:128 * t + rows, :], in_=ysb[0:rows, :])

        # ---- phase 4: banded blocks (S^T formulation) ----
        for b in range(NB):
            mv = 0 if b == 0 else (2 if b == NB - 1 else 1)
            st1 = [ps_st.tile([128, 384], F32, tag=f"st1{i}", name=f"st1{i}")
                   for i in range(2)]
            st2 = [ps_st.tile([33, 384], F32, tag=f"st2{i}", name=f"st2{i}")
                   for i in range(2)]
            for h in range(6):
                g, hh = h // 3, h % 3
                qs = QT[h][0:64, FP + 1 + 128 * b: FP + 129 + 128 * b]
                # S^T chunk 1: keys (-15..112 rel block) on partitions
                nc.tensor.matmul(
                    st1[g][:, 128 * hh:128 * hh + 128],
                    lhsT=KT[h][0:64, 128 * b + 1: 128 * b + 129],
                    rhs=qs, start=True, stop=True,
                )
                # S^T chunk 2: tail keys 113..144 (32 rows)
                nc.tensor.matmul(
                    st2[g][0:32, 128 * hh:128 * hh + 128],
                    lhsT=KT[h][0:64, 128 * b + 129: 128 * b + 161],
                    rhs=qs, start=True, stop=True,
                )
                # cls key -> row 32
                nc.tensor.matmul(
                    st2[g][32:33, 128 * hh:128 * hh + 128],
                    lhsT=KT[h][0:64, FP:FP + 1],
                    rhs=qs, start=True, stop=True,
                )
            a_e1 = apool.tile([128, 768], BF, tag="a_e1", name="a_e1")
            a_e2 = apool.tile([33, 768], BF, tag="a_e2", name="a_e2")
            for g in range(2):
                nc.scalar.activation(out=a_e1[:, 384 * g:384 * g + 384],
                                     in_=st1[g][:], func=AF.Exp)
                nc.scalar.activation(out=a_e2[:, 384 * g:384 * g + 384],
                                     in_=st2[g][:], func=AF.Exp)
            am1 = apool.tile([128, 768], BF, tag="am1", name="am1")
            am2 = apool.tile([33, 768], BF, tag="am2", name="am2")
            nc.vector.tensor_mul(am1[:], a_e1[:], mk1_sb[mv][:])
            nc.vector.tensor_mul(am2[:], a_e2[:], mk2_sb[mv][:])
            # AV: natural-layout output + fused row sums (ones cols)
            po_nat = po_tile()
            for h in range(6):
                nc.tensor.matmul(
                    po_nat[:, 65 * h:65 * h + 65],
                    lhsT=am1[:, 128 * h:128 * h + 128],
                    rhs=Vpp[b][:, 65 * h:65 * h + 65],
                    start=True, stop=False,
                )
                nc.tensor.matmul(
                    po_nat[:, 65 * h:65 * h + 65],
                    lhsT=am2[0:33, 128 * h:128 * h + 128],
                    rhs=Vtail[b][0:33, 65 * h:65 * h + 65],
                    start=False, stop=True,
                )
            o_u = opool.tile([128, VW], BF, tag="o_u", name="o_u")
            nc.vector.tensor_copy(o_u[:], po_nat[:, 0:VW])
            recips = spool.tile([128, 6], F32, tag="recips", name="recips")
            for h in range(6):
                nc.vector.reciprocal(recips[:, h:h + 1],
                                     po_nat[:, 65 * h + 64:65 * h + 65])
            o_n = opool.tile([128, E], BF, tag="o_n", name="o_n")
            for h in range(6):
                nc.vector.tensor_scalar_mul(
                    o_n[:, 64 * h:64 * h + 64], o_u[:, 65 * h:65 * h + 64],
                    recips[:, h:h + 1],
                )
            for j in range(3):
                pt = pt_pp[(b * 3 + j) % 2]
                nc.tensor.transpose(
                    pt[0:128, 0:128], o_n[:, 128 * j:128 * j + 128], ident_sb[:]
                )
                nc.any.tensor_copy(
                    OT[j][:, 1 + 128 * b: 129 + 128 * b], pt[:, 0:128]
                )
            if b == 0:
                cls_scores(0)
                cls_scores(1)
                cls_scores(2)
            elif b == 1:
                cls_gather()
                cls_transposes()
            elif b == 2:
                cls_finish()
            elif b == 3:
                emit_out_tile(0)
                emit_out_tile(1)
            elif b == 4:
                emit_out_tile(2)
                emit_out_tile(3)
            elif b == 5:
                emit_out_tile(4)
                emit_out_tile(5)
            elif b == 6:
                emit_out_tile(6)
            elif b == 7:
                emit_out_tile(7)
                emit_out_tile(8)

    nc.compile()
    return nc


_CACHE = {}


def get_nc():
    if "nc" not in _CACHE:
        nc = bacc.Bacc("TRN2", target_bir_lowering=False, debug=False)
        _CACHE["nc"] = build_program(nc)
    return _CACHE["nc"]


def kernel(x, Wq, bq, Wk, bk, Wv, bv, Wo, bo, _trace=False):
    from concourse.bass_utils import run_bass_kernel_spmd

    x = np.asarray(x)
    in_maps = [
        host_inputs(x[b], Wq, bq, Wk, bk, Wv, bv, Wo, bo) for b in range(B)
    ]
    nc = get_nc()
    res = run_bass_kernel_spmd(nc, in_maps, core_ids=list(range(8)), trace=_trace)
    out = np.stack(
        [np.asarray(res.results[b]["out"], dtype=np.float32) for b in range(B)],
        axis=0,
    )
    if _trace:
        return out, res
    return out
